# revision 31
# baseline (speedup 1.0000x reference)
"""Trainium2 Bass kernel for nn_Biaffine (B=4, S=512, D=512, R=64).

Math: the reference computes
    left = einsum('bxi,irj,byj->bxyr', hf, U1, hb)
    out  = mean_y(left + rf[:, :, None] + rb[:, None] + bias)
The mean over y commutes with everything:
    mean_y(left)[b,x,r] = sum_ij hf[b,x,i] U1[i,r,j] hbbar[b,j],
    hbbar = mean_y(hb).
So out[b,x,r] = sum_i hf[b,x,i] * (V[b,i,r] + U2a[i,r]) + rbbar[b,r] + bias[r]
with V[b,i,r] = sum_j U1[i,r,j] hbbar[b,j], rbbar = hbbar @ U2b.

Sharding: tensor-parallel over r (dep_vec_dim): core c owns r in [8c, 8c+8).
Each core reads its U1 shard and full hf; hb's mean is y-sharded and combined
with an 8KB on-chip AllReduce. Each core computes out[:, :, 8c:8c+8]; the
host concatenates.

Precision/traffic: the kernel is HBM-byte-bound, so the big tensors are
narrowed on the host: U1 ships as fp8 e3m4 (2MB/core) and hf/hb as bf16
(2MB + 0.25MB/core), leaving ~4.5MB/core vs 14.6MB for the all-fp32 version.
Power-of-2 scale folds keep every staged value inside e3m4's normal range at
zero device cost, and the final rel err is ~4.4e-3 against the 2e-2 gate.

Engine layout per rep: the V matmuls run as two 4-way column-tiled quads
(jc-outer issue order so the 4 col-group chains stream concurrently), the
[b,i]->[i,b] flip is 4 full-width PE transposes into one PSUM tile per quad
followed by a single strided DVE add, the PSUM->SBUF copies ride the ACT
engine, and rbbar+bias re-enters via a per-partition ACT bias column. The
out matmuls are column-tiled over the 4 batches at PSUM partitions 32b.
"""

import os
import sys

import numpy as np

try:
    import concourse.bass as bass  # noqa: F401
except ImportError:  # pragma: no cover
    sys.path.insert(0, "/opt/trn_rl_repo")

B, S, D, R = 4, 512, 512, 64
NCORES = 8
RB = R // NCORES  # 8 r's per core
P = 128
JC = D // P  # 4 j-chunks
IC = D // P  # 4 i-chunks
SY = S // NCORES  # 64 y's per core (mean partial, AllReduce'd)

# U1 is streamed in fp8 (e3m4: 4 mantissa bits). Power-of-2 scale folding
# keeps every staged value in fp8's normal range with zero extra device ops:
#   u1' = 1024*U1 (fp8), hb' = hb/8 (bf16)  =>  V' = 128*S*V
#   u2a' = 128*S*U2a, hft' = hf/(128*S)     =>  out = hf @ (V + U2a)  exactly
#   u2b' = 8*U2b/S                          =>  rbbar path unscaled
U1_SCALE = 1024.0
HB_SCALE = 2.0          # hb staged in fp8 e3m4 at x2 (normal range)
CAST_SCALE = 1.0 / 16.0  # on-chip hbbar->fp8 cast rescale (ACT, free)
VSCALE = U1_SCALE * HB_SCALE * CAST_SCALE * S  # 128*S, unchanged

# module-level knobs / results (test.py uses these; harness doesn't need them)
TRACE = os.environ.get("BASS_KERNEL_TRACE", "0") == "1"
LAST_RESULTS = None

_NC_CACHE = {}


def _build_nc(n_repeat=1, solo_ar=False, mode='full'):
    import concourse.bacc as bacc
    import concourse.mybir as mybir
    import concourse.tile as tile
    from concourse.masks import make_identity
    fp32 = mybir.dt.float32
    bf16 = mybir.dt.bfloat16
    fp8 = mybir.dt.float8e3

    nc = bacc.Bacc("TRN2", target_bir_lowering=False, debug=False, num_devices=NCORES)

    hft_d = nc.dram_tensor("hft", [B, D, S], bf16, kind="ExternalInput")
    hb_d = nc.dram_tensor("hb", [D, B, SY], fp8, kind="ExternalInput")
    u1t_d = nc.dram_tensor("u1t", [D, RB, D], fp8, kind="ExternalInput")
    u2t_d = nc.dram_tensor("u2t", [P, IC, 2 * RB], fp32, kind="ExternalInput")
    bias_d = nc.dram_tensor("biasr", [1, RB], fp32, kind="ExternalInput")
    out_d = nc.dram_tensor("out", [B, RB, S], fp32, kind="ExternalOutput")

    with tile.TileContext(nc) as tc:
        with (
            tc.tile_pool(name="const", bufs=1) as cpool,
            tc.tile_pool(name="data", bufs=1) as dpool,
            tc.tile_pool(name="psum", bufs=8, space="PSUM") as ppool,
            tc.tile_pool(name="dram", bufs=1, space="DRAM") as drpool,
        ):
            identity_sq = cpool.tile([P, P], fp32, tag="identity_sq")
            make_identity(nc, identity_sq)
            ones1 = cpool.tile([1, S], fp32, tag="ones1")
            nc.vector.memset(ones1, 1.0)

            for _rep in range(n_repeat):
                _emit_body(
                    nc, dpool, ppool, drpool, fp32, ones1, identity_sq,
                    hft_d, hb_d, u1t_d, u2t_d, bias_d, out_d, solo_ar,
                    mode=mode,
                )

    nc.compile()
    return nc


def _emit_body(
    nc, dpool, ppool, drpool, fp32, ones1, identity_sq,
    hft_d, hb_d, u1t_d, u2t_d, bias_d, out_d, solo_ar=False, mode="full",
):
    import concourse.mybir as mybir
    bf16 = mybir.dt.bfloat16
    fp8 = mybir.dt.float8e3
    if True:
        if True:
            u2sb = dpool.tile([P, IC, 2 * RB], fp32, tag="u2sb", bufs=2)
            bias_sb = dpool.tile([1, RB], fp32, tag="bias_sb", bufs=2)
            hbbarT = dpool.tile([P, JC * B], fp32, tag="hbbarT", bufs=2)
            hbbarT_bf = dpool.tile([P, JC * B], fp8, tag="hbbarT_bf", bufs=2)
            vass = dpool.tile([P, IC, B, RB], bf16, tag="vass", bufs=2)

            # --- hb y-slice load FIRST: it gates the hbbar reduce -> AR
            # chain that everything else waits on. Host-transposed to
            # [j, b, y] so the partial mean is a DVE free-axis reduce.
            hbt = dpool.tile([P, JC, B, SY], fp8, tag="hb", bufs=2)
            nc.sync.dma_start(
                out=hbt, in_=hb_d.ap().rearrange("(jc p) b y -> p jc b y", p=P)
            )

            # --- big loads issued up-front: the SP DGE queue is in-order, so
            # no DMA with a semaphore wait may precede these (head-of-line).
            # u1 bufs=1 is safe: V(i) finishes before hft(i) drains, so the
            # slot-free wait for u1t(i+1) never starves the rings.
            u1_pairs = []
            for jp in range(JC // 2):
                u1t_t = dpool.tile([P, 2, RB, D], fp8, tag=f"u1_{jp}", bufs=2)
                nc.sync.dma_start(
                    out=u1t_t,
                    in_=u1t_d.ap()[jp * 2 * P : (jp + 1) * 2 * P].rearrange(
                        "(j p) r i -> p j r i", p=P
                    ),
                )
                u1_pairs.append(u1t_t)
            u1_tiles = [u1_pairs[jc // 2][:, jc % 2] for jc in range(JC)]
            hft_pairs = []
            for bp in range(B // 2):
                hft_t = dpool.tile([P, 2, IC, S], bf16, tag=f"hft{bp}", bufs=2)
                nc.sync.dma_start(
                    out=hft_t,
                    in_=hft_d.ap()[bp * 2 : bp * 2 + 2].rearrange(
                        "b (ic p) x -> p b ic x", p=P
                    ),
                )
                hft_pairs.append(hft_t)
            hft_tiles = [hft_pairs[b // 2][:, b % 2] for b in range(B)]

            # small late-consumed inputs ride behind the big loads
            nc.sync.dma_start(out=u2sb, in_=u2t_d.ap())
            nc.sync.dma_start(out=bias_sb, in_=bias_d.ap())

            if mode == "dma":
                # consume each loaded tile with a cheap DVE op so reps chain
                outd = dpool.tile([P, 16], fp32, tag="dmaout", bufs=2)
                for jc in range(JC):
                    nc.vector.reduce_sum(
                        outd[:, jc : jc + 1],
                        u1_tiles[jc][:, 0, :32].rearrange("p a -> p a"),
                        axis=mybir.AxisListType.X,
                    )
                for b in range(B):
                    nc.vector.reduce_sum(
                        outd[:, 4 + b : 5 + b],
                        hft_tiles[b][:, 0, :32],
                        axis=mybir.AxisListType.X,
                    )
                nc.vector.reduce_sum(
                    outd[:, 8:9], hbt[:, 0, 0, :32], axis=mybir.AxisListType.X
                )
                nc.scalar.dma_start(out=out_d.ap()[0, :1, :16], in_=outd[:1, :])
                return

            # partial hbbarT[j, b] = sum_{y in slice} hb[b, y, j] (unscaled;
            # U1T/U2b carry the 1/S), via one DVE free-axis reduce
            hbbarT_part = dpool.tile([P, JC * B], fp32, tag="hbbarT_part", bufs=2)
            for jc in range(JC):
                nc.vector.reduce_sum(
                    hbbarT_part[:, jc * B : (jc + 1) * B],
                    hbt[:, jc],
                    axis=mybir.AxisListType.X,
                )

            # --- AllReduce the 8KB partial means across the 8 cores ---
            ar_in = drpool.tile([P, JC * B], fp32, tag="ar_in", bufs=2)
            ar_out = drpool.tile([P, JC * B], fp32, tag="ar_out", bufs=2)
            nc.scalar.dma_start(out=ar_in[:], in_=hbbarT_part)
            nc.gpsimd.collective_compute(
                "AllReduce",
                mybir.AluOpType.add,
                replica_groups=(
                    [[c] for c in range(NCORES)] if solo_ar
                    else [list(range(NCORES))]
                ),
                ins=[ar_in.opt()],
                outs=[ar_out.opt()],
            )
            nc.scalar.dma_start(out=hbbarT, in_=ar_out[:])
            # fp8 copy of hbbar for the fp8 V-matmuls (rbbar stays fp32);
            # the 1/16 rescale keeps the cast inside e3m4's normal range
            nc.scalar.mul(out=hbbarT_bf, in_=hbbarT, mul=CAST_SCALE)

            # --- V[b, i] per r: hbbarT stationary (LDW = 4 cols), U1 streams
            # as the N=512 moving operand. Four r's share one PSUM tile at
            # base partitions {0,32,64,96} (legal tile_position[1] for M=4),
            # so the [b, i] -> [i, b] PE transposes drop from 32 to 8.
            # jc-outer / k-inner issue order: the 4 col-group chains start
            # ~4ns apart and stream concurrently (pc-monotone starts would
            # serialize them under k-outer order). Both quads' matmuls are
            # emitted before any transpose so the PE never waits on the DVE.
            ps_qs = []
            for rq in range(RB // 4):
                ps_q = ppool.tile([P, 512], fp32, tag="ps")
                for jc in range(JC):
                    for k in range(4):
                        r = rq * 4 + k
                        nc.tensor.matmul(
                            ps_q[k * 32 : k * 32 + B, :D],
                            hbbarT_bf[:, jc * B : (jc + 1) * B],
                            u1_tiles[jc][:, r, :],
                            start=(jc == 0),
                            stop=(jc == JC - 1),
                            tile_position=(0, k * 32),
                        )
                ps_qs.append(ps_q)
            for rq in range(RB // 4):
                vq = dpool.tile([P, D], fp32, tag="vq", bufs=2)
                nc.scalar.copy(out=vq, in_=ps_qs[rq][:P, :D])
                # 4 full-width transposes land in ONE PSUM tile; a single
                # strided add then moves all (ic, b, r-quad) slices at once:
                # ps_tq col ic*128 + k*32 + b, viewed [p, ic, k, b] -> vass
                ps_tq = ppool.tile([P, 512], fp32, tag="ps")
                for ic in range(IC):
                    nc.tensor.transpose(
                        ps_tq[:P, ic * P : (ic + 1) * P],
                        vq[:, ic * P : (ic + 1) * P],
                        identity_sq,
                    )
                nc.vector.tensor_tensor(
                    out=vass[:, :, :, rq * 4 : (rq + 1) * 4],
                    in0=ps_tq.rearrange("p (ic k c) -> p ic k c", ic=IC, k=4)[
                        :, :, :, :B
                    ].rearrange("p ic k b -> p ic b k"),
                    in1=u2sb[:, :, None, rq * 4 : (rq + 1) * 4].to_broadcast(
                        (P, IC, B, 4)
                    ),
                    op=mybir.AluOpType.add,
                )

            # --- rbbar[b, r] + bias as 4 M=1 rows on partition 0 (cols
            # b*32+r), emitted HERE so these tiny matmuls run in the PE
            # bubble between the V transposes and the hft-gated out phase
            # (their result is only consumed by the final ACT bias) ---
            ps_rb = ppool.tile([P, 512], fp32, tag="ps")
            for b in range(B):
                for jc in range(JC):
                    nc.tensor.matmul(
                        ps_rb[:1, b * 32 : b * 32 + RB],
                        hbbarT[:, jc * B + b : jc * B + b + 1],
                        u2sb[:, jc, RB : 2 * RB],
                        start=(jc == 0),
                        stop=False,
                    )
                nc.tensor.matmul(
                    ps_rb[:1, b * 32 : b * 32 + RB],
                    ones1[:1, :1],
                    bias_sb,
                    start=False,
                    stop=True,
                )
            rbbF = dpool.tile([1, B * 32], fp32, tag="rbbF", bufs=2)
            nc.vector.tensor_copy(out=rbbF, in_=ps_rb[:1, : B * 32])
            # transpose the row to a per-partition column [128, 1] so the
            # final PSUM->SBUF copy can fold it in as an ACT bias
            ps_rt = ppool.tile([P, 512], fp32, tag="ps")
            nc.tensor.transpose(ps_rt[:P, :1], rbbF, identity_sq[:1, :1])
            rbbC = dpool.tile([P, 1], fp32, tag="rbbC", bufs=2)
            nc.vector.tensor_copy(out=rbbC, in_=ps_rt[:P, :1])

            # --- out[r, x]: contract i. The 4 b's live at PSUM base
            # partitions {0,32,64,96} (col-tiled, concurrent chains); a
            # trailing K=1 ones-matmul folds in rbbar+bias per group.
            ps_o = ppool.tile([P, 512], fp32, tag="ps")
            for ic in range(IC):
                for b in range(B):
                    nc.tensor.matmul(
                        ps_o[b * 32 : b * 32 + RB, :S],
                        vass[:, ic, b, :],
                        hft_tiles[b][:, ic, :],
                        start=(ic == 0),
                        stop=(ic == IC - 1),
                        tile_position=(0, b * 32),
                    )
            out_sb = dpool.tile([P, S], fp32, tag="out_sb", bufs=2)
            nc.scalar.activation(
                out=out_sb,
                in_=ps_o,
                func=mybir.ActivationFunctionType.Identity,
                bias=rbbC,
            )
            for b in range(B):
                nc.scalar.dma_start(
                    out=out_d.ap()[b], in_=out_sb[b * 32 : b * 32 + RB]
                )


def _get_nc(n_repeat=1):
    if n_repeat not in _NC_CACHE:
        _NC_CACHE[n_repeat] = _build_nc(n_repeat)
    return _NC_CACHE[n_repeat]


def _prep_inputs(h_forward, h_backward, U_1, U_2, bias):
    import ml_dtypes

    bf16 = ml_dtypes.bfloat16
    f8 = ml_dtypes.float8_e3m4
    hf = np.ascontiguousarray(np.asarray(h_forward, dtype=np.float32))
    hb = np.ascontiguousarray(np.asarray(h_backward, dtype=np.float32))
    u1 = np.asarray(U_1, dtype=np.float32)
    u2 = np.asarray(U_2, dtype=np.float32)
    bz = np.asarray(bias, dtype=np.float32)

    hft = np.ascontiguousarray(
        (hf.transpose(0, 2, 1) * np.float32(1.0 / VSCALE)).astype(bf16)
    )  # [B, i, x], scaled so out = hft' @ (VSCALE*(V+U2a)) is exact

    in_maps = []
    for c in range(NCORES):
        rs = slice(c * RB, (c + 1) * RB)
        u1t_c = np.ascontiguousarray(
            (u1[:, rs, :].transpose(2, 1, 0) * np.float32(U1_SCALE)).astype(f8)
        )  # [j, r, i], fp8 with power-of-2 gain to dodge denormals
        # pre-packed u2sb layout [d%P, dchunk, 2*RB]: cols 0:RB = VSCALE*U2a,
        # RB:2RB = U2b/(S*HB_SCALE) (so both match their scaled partners)
        u2t_c = np.ascontiguousarray(
            np.concatenate(
                [
                    u2[:D, rs].reshape(IC, P, RB).transpose(1, 0, 2)
                    * np.float32(VSCALE),
                    u2[D:, rs].reshape(IC, P, RB).transpose(1, 0, 2)
                    * np.float32(1.0 / (S * HB_SCALE)),
                ],
                axis=2,
            )
        )
        bias_c = np.ascontiguousarray(bz[rs].reshape(1, RB))
        hb_c = np.ascontiguousarray(
            (hb[:, c * SY : (c + 1) * SY, :].transpose(2, 0, 1)
             * np.float32(HB_SCALE)).astype(f8)
        )  # [D(j), B, SY], fp8 e3m4 at x2 (normal range)
        in_maps.append(
            {
                "hft": hft,
                "hb": hb_c,
                "u1t": u1t_c,
                "u2t": u2t_c,
                "biasr": bias_c,
            }
        )
    return in_maps


def _get_exec():
    """One jitted sharded executable, cached for the process lifetime.

    Repeated kernel() calls reuse it — re-jitting a second executable with
    collectives in the same process has been observed to wedge the NRT
    (NRT_EXEC_UNIT_UNRECOVERABLE), while re-executing one executable is solid.
    """
    if "exec" in _EXEC_CACHE:
        return _EXEC_CACHE["exec"]

    import jax
    from jax.sharding import Mesh, PartitionSpec

    import warnings

    with warnings.catch_warnings():
        warnings.simplefilter("ignore")
        from jax.experimental.shard_map import shard_map

    from concourse import mybir
    from concourse.bass2jax import (
        _bass_exec_p,
        install_neuronx_cc_hook,
        partition_id_tensor,
    )

    install_neuronx_cc_hook()
    nc = _get_nc()
    partition_name = nc.partition_id_tensor.name if nc.partition_id_tensor else None
    in_names, out_names, out_avals = [], [], []
    for alloc in nc.m.functions[0].allocations:
        if not isinstance(alloc, mybir.MemoryLocationSet):
            continue
        name = alloc.memorylocations[0].name
        if alloc.kind == "ExternalInput":
            if name != partition_name:
                in_names.append(name)
        elif alloc.kind == "ExternalOutput":
            out_names.append(name)
            out_avals.append(
                jax.core.ShapedArray(tuple(alloc.tensor_shape), mybir.dt.np(alloc.dtype))
            )
    all_names = in_names + out_names
    if partition_name is not None:
        all_names = all_names + [partition_name]

    def _body(*args):
        operands = list(args)
        if partition_name is not None:
            operands.append(partition_id_tensor())
        return tuple(
            _bass_exec_p.bind(
                *operands,
                out_avals=tuple(out_avals),
                in_names=tuple(all_names),
                out_names=tuple(out_names),
                lowering_input_output_aliases=(),
                sim_require_finite=True,
                sim_require_nnan=True,
                nc=nc,
            )
        )

    devices = jax.devices()[:NCORES]
    mesh = Mesh(np.asarray(devices), ("core",))
    n_args = len(in_names) + len(out_avals)
    fn = jax.jit(
        shard_map(
            _body,
            mesh=mesh,
            in_specs=(PartitionSpec("core"),) * n_args,
            out_specs=(PartitionSpec("core"),) * len(out_names),
            check_rep=False,
        ),
        keep_unused=True,
    )
    sh = jax.sharding.NamedSharding(mesh, PartitionSpec("core"))
    _EXEC_CACHE["exec"] = (fn, sh, in_names, out_names, out_avals)
    return _EXEC_CACHE["exec"]


_EXEC_CACHE = {}


def kernel(h_forward, h_backward, U_1, U_2, bias):
    import jax

    fn, sh, in_names, out_names, out_avals = _get_exec()
    in_maps = _prep_inputs(h_forward, h_backward, U_1, U_2, bias)
    args = [
        jax.device_put(
            np.concatenate([in_maps[c][name] for c in range(NCORES)], axis=0), sh
        )
        for name in in_names
    ]
    for av in out_avals:
        args.append(
            jax.device_put(
                np.zeros((NCORES * av.shape[0], *av.shape[1:]), av.dtype), sh
            )
        )
    out_arrs = fn(*args)
    oi = out_names.index("out")
    full = np.asarray(out_arrs[oi]).reshape(NCORES, B, RB, S)  # [core, B, RB, S]
    out = np.concatenate(list(full), axis=1)  # [B, R, S]
    return np.ascontiguousarray(out.transpose(0, 2, 1))  # [B, S, R]



# revision 32
# speedup vs baseline: 1.3766x; 1.3766x over previous
"""Trainium2 Bass kernel for nn_Biaffine (B=4, S=512, D=512, R=64).

Math: the reference computes
    left = einsum('bxi,irj,byj->bxyr', hf, U1, hb)
    out  = mean_y(left + rf[:, :, None] + rb[:, None] + bias)
The mean over y commutes with everything:
    mean_y(left)[b,x,r] = sum_ij hf[b,x,i] U1[i,r,j] hbbar[b,j],
    hbbar = mean_y(hb).
So out[b,x,r] = sum_i hf[b,x,i] * (V[b,i,r] + U2a[i,r]) + rbbar[b,r] + bias[r]
with V[b,i,r] = sum_j U1[i,r,j] hbbar[b,j], rbbar = hbbar @ U2b.

Sharding: tensor-parallel over r (dep_vec_dim): core c owns r in [8c, 8c+8).
Each core reads its U1 shard and full hf; hb's mean is y-sharded and combined
with an 8KB on-chip AllReduce. Each core computes out[:, :, 8c:8c+8]; the
host concatenates.

Precision/traffic: the kernel is HBM-byte-bound, so the big tensors are
narrowed on the host: U1 ships as fp8 e3m4 (2MB/core) and hf/hb as bf16
(2MB + 0.25MB/core), leaving ~4.5MB/core vs 14.6MB for the all-fp32 version.
Power-of-2 scale folds keep every staged value inside e3m4's normal range at
zero device cost, and the final rel err is ~4.4e-3 against the 2e-2 gate.

Engine layout per rep: the V matmuls run as two 4-way column-tiled quads
(jc-outer issue order so the 4 col-group chains stream concurrently), the
[b,i]->[i,b] flip is 4 full-width PE transposes into one PSUM tile per quad
followed by a single strided DVE add, the PSUM->SBUF copies ride the ACT
engine, and rbbar+bias re-enters via a per-partition ACT bias column. The
out matmuls are column-tiled over the 4 batches at PSUM partitions 32b.
"""

import os
import sys

import numpy as np

try:
    import concourse.bass as bass  # noqa: F401
except ImportError:  # pragma: no cover
    sys.path.insert(0, "/opt/trn_rl_repo")

B, S, D, R = 4, 512, 512, 64
NCORES = 8
RB = R // NCORES  # 8 r's per core
P = 128
JC = D // P  # 4 j-chunks
IC = D // P  # 4 i-chunks
SY = S // NCORES  # 64 y's per core (mean partial, AllReduce'd)

# U1 is streamed in fp8 (e3m4: 4 mantissa bits). Power-of-2 scale folding
# keeps every staged value in fp8's normal range with zero extra device ops:
#   u1' = 1024*U1 (fp8), hb' = hb/8 (bf16)  =>  V' = 128*S*V
#   u2a' = 128*S*U2a, hft' = hf/(128*S)     =>  out = hf @ (V + U2a)  exactly
#   u2b' = 8*U2b/S                          =>  rbbar path unscaled
U1_SCALE = 1024.0
HB_SCALE = 2.0          # hb staged in fp8 e3m4 at x2 (normal range)
CAST_SCALE = 1.0 / 16.0  # on-chip hbbar->fp8 cast rescale (ACT, free)
VSCALE = U1_SCALE * HB_SCALE * CAST_SCALE * S  # 128*S, unchanged

# module-level knobs / results (test.py uses these; harness doesn't need them)
TRACE = os.environ.get("BASS_KERNEL_TRACE", "0") == "1"
LAST_RESULTS = None

_NC_CACHE = {}


def _build_nc(n_repeat=1, solo_ar=False, mode='full'):
    import concourse.bacc as bacc
    import concourse.mybir as mybir
    import concourse.tile as tile
    from concourse.masks import make_identity
    fp32 = mybir.dt.float32
    bf16 = mybir.dt.bfloat16
    fp8 = mybir.dt.float8e3

    nc = bacc.Bacc("TRN2", target_bir_lowering=False, debug=False, num_devices=NCORES)

    hft_d = nc.dram_tensor("hft", [B, D, S], bf16, kind="ExternalInput")
    hb_d = nc.dram_tensor("hb", [D, B, SY], fp8, kind="ExternalInput")
    u1t_d = nc.dram_tensor("u1t", [D, RB, D], fp8, kind="ExternalInput")
    u2t_d = nc.dram_tensor("u2t", [P, IC, 2 * RB], fp32, kind="ExternalInput")
    bias_d = nc.dram_tensor("biasr", [1, RB], fp32, kind="ExternalInput")
    out_d = nc.dram_tensor("out", [B, RB, S], fp32, kind="ExternalOutput")

    with tile.TileContext(nc) as tc:
        with (
            tc.tile_pool(name="const", bufs=1) as cpool,
            tc.tile_pool(name="data", bufs=1) as dpool,
            tc.tile_pool(name="psum", bufs=8, space="PSUM") as ppool,
            tc.tile_pool(name="dram", bufs=1, space="DRAM") as drpool,
        ):
            identity_sq = cpool.tile([P, P], fp32, tag="identity_sq")
            make_identity(nc, identity_sq)
            ones1 = cpool.tile([1, S], fp32, tag="ones1")
            nc.vector.memset(ones1, 1.0)

            for _rep in range(n_repeat):
                _emit_body(
                    nc, dpool, ppool, drpool, fp32, ones1, identity_sq,
                    hft_d, hb_d, u1t_d, u2t_d, bias_d, out_d, solo_ar,
                    mode=mode,
                )

    nc.compile()
    return nc


def _emit_body(
    nc, dpool, ppool, drpool, fp32, ones1, identity_sq,
    hft_d, hb_d, u1t_d, u2t_d, bias_d, out_d, solo_ar=False, mode="full",
):
    import concourse.mybir as mybir
    bf16 = mybir.dt.bfloat16
    fp8 = mybir.dt.float8e3
    if True:
        if True:
            u2sb = dpool.tile([P, IC, 2 * RB], fp32, tag="u2sb", bufs=3)
            bias_sb = dpool.tile([1, RB], fp32, tag="bias_sb", bufs=3)
            hbbarT = dpool.tile([P, JC * B], fp32, tag="hbbarT", bufs=3)
            hbbarT_bf = dpool.tile([P, JC * B], fp8, tag="hbbarT_bf", bufs=3)
            vass = dpool.tile([P, IC, B, RB], bf16, tag="vass", bufs=3)

            # --- hb y-slice load FIRST: it gates the hbbar reduce -> AR
            # chain that everything else waits on. Host-transposed to
            # [j, b, y] so the partial mean is a DVE free-axis reduce.
            hbt = dpool.tile([P, JC, B, SY], fp8, tag="hb", bufs=3)
            nc.sync.dma_start(
                out=hbt, in_=hb_d.ap().rearrange("(jc p) b y -> p jc b y", p=P)
            )

            # --- big loads issued up-front: the SP DGE queue is in-order, so
            # no DMA with a semaphore wait may precede these (head-of-line).
            # u1 bufs=1 is safe: V(i) finishes before hft(i) drains, so the
            # slot-free wait for u1t(i+1) never starves the rings.
            u1_pairs = []
            for jp in range(JC // 2):
                u1t_t = dpool.tile([P, 2, RB, D], fp8, tag=f"u1_{jp}", bufs=3)
                nc.sync.dma_start(
                    out=u1t_t,
                    in_=u1t_d.ap()[jp * 2 * P : (jp + 1) * 2 * P].rearrange(
                        "(j p) r i -> p j r i", p=P
                    ),
                )
                u1_pairs.append(u1t_t)
            u1_tiles = [u1_pairs[jc // 2][:, jc % 2] for jc in range(JC)]
            hft_pairs = []
            for bp in range(B // 2):
                hft_t = dpool.tile([P, 2, IC, S], bf16, tag=f"hft{bp}", bufs=3)
                nc.sync.dma_start(
                    out=hft_t,
                    in_=hft_d.ap()[bp * 2 : bp * 2 + 2].rearrange(
                        "b (ic p) x -> p b ic x", p=P
                    ),
                )
                hft_pairs.append(hft_t)
            hft_tiles = [hft_pairs[b // 2][:, b % 2] for b in range(B)]

            # small late-consumed inputs ride behind the big loads
            nc.sync.dma_start(out=u2sb, in_=u2t_d.ap())
            nc.sync.dma_start(out=bias_sb, in_=bias_d.ap())

            if mode == "dma":
                # consume each loaded tile with a cheap DVE op so reps chain
                outd = dpool.tile([P, 16], fp32, tag="dmaout", bufs=2)
                for jc in range(JC):
                    nc.vector.reduce_sum(
                        outd[:, jc : jc + 1],
                        u1_tiles[jc][:, 0, :32].rearrange("p a -> p a"),
                        axis=mybir.AxisListType.X,
                    )
                for b in range(B):
                    nc.vector.reduce_sum(
                        outd[:, 4 + b : 5 + b],
                        hft_tiles[b][:, 0, :32],
                        axis=mybir.AxisListType.X,
                    )
                nc.vector.reduce_sum(
                    outd[:, 8:9], hbt[:, 0, 0, :32], axis=mybir.AxisListType.X
                )
                nc.scalar.dma_start(out=out_d.ap()[0, :1, :16], in_=outd[:1, :])
                return

            # partial hbbarT[j, b] = sum_{y in slice} hb[b, y, j] (unscaled;
            # U1T/U2b carry the 1/S), via one DVE free-axis reduce
            hbbarT_part = dpool.tile([P, JC * B], fp32, tag="hbbarT_part", bufs=3)
            for jc in range(JC):
                nc.vector.reduce_sum(
                    hbbarT_part[:, jc * B : (jc + 1) * B],
                    hbt[:, jc],
                    axis=mybir.AxisListType.X,
                )

            # --- AllReduce the 8KB partial means across the 8 cores ---
            ar_in = drpool.tile([P, JC * B], fp32, tag="ar_in", bufs=3)
            ar_out = drpool.tile([P, JC * B], fp32, tag="ar_out", bufs=3)
            nc.scalar.dma_start(out=ar_in[:], in_=hbbarT_part)
            nc.gpsimd.collective_compute(
                "AllReduce",
                mybir.AluOpType.add,
                replica_groups=(
                    [[c] for c in range(NCORES)] if solo_ar
                    else [list(range(NCORES))]
                ),
                ins=[ar_in.opt()],
                outs=[ar_out.opt()],
            )
            nc.scalar.dma_start(out=hbbarT, in_=ar_out[:])
            # fp8 copy of hbbar for the fp8 V-matmuls (rbbar stays fp32);
            # the 1/16 rescale keeps the cast inside e3m4's normal range
            nc.scalar.mul(out=hbbarT_bf, in_=hbbarT, mul=CAST_SCALE)

            # --- V[b, i] per r: hbbarT stationary (LDW = 4 cols), U1 streams
            # as the N=512 moving operand. Four r's share one PSUM tile at
            # base partitions {0,32,64,96} (legal tile_position[1] for M=4),
            # so the [b, i] -> [i, b] PE transposes drop from 32 to 8.
            # jc-outer / k-inner issue order: the 4 col-group chains start
            # ~4ns apart and stream concurrently (pc-monotone starts would
            # serialize them under k-outer order). Both quads' matmuls are
            # emitted before any transpose so the PE never waits on the DVE.
            ps_qs = []
            for rq in range(RB // 4):
                ps_q = ppool.tile([P, 512], fp32, tag="ps")
                for jc in range(JC):
                    for k in range(4):
                        r = rq * 4 + k
                        nc.tensor.matmul(
                            ps_q[k * 32 : k * 32 + B, :D],
                            hbbarT_bf[:, jc * B : (jc + 1) * B],
                            u1_tiles[jc][:, r, :],
                            start=(jc == 0),
                            stop=(jc == JC - 1),
                            tile_position=(0, k * 32),
                        )
                ps_qs.append(ps_q)
            for rq in range(RB // 4):
                vq = dpool.tile([P, D], fp32, tag="vq", bufs=3)
                nc.scalar.copy(out=vq, in_=ps_qs[rq][:P, :D])
                # 4 full-width transposes land back in the quad's own PSUM
                # tile (dead once the vq copy has read it -- the WAR dep
                # keeps ordering); a single strided add then moves all
                # (ic, b, r-quad) slices: col ic*128 + k*32 + b
                ps_tq = ps_qs[rq]
                for ic in range(IC):
                    nc.tensor.transpose(
                        ps_tq[:P, ic * P : (ic + 1) * P],
                        vq[:, ic * P : (ic + 1) * P],
                        identity_sq,
                    )
                nc.vector.tensor_tensor(
                    out=vass[:, :, :, rq * 4 : (rq + 1) * 4],
                    in0=ps_tq.rearrange("p (ic k c) -> p ic k c", ic=IC, k=4)[
                        :, :, :, :B
                    ].rearrange("p ic k b -> p ic b k"),
                    in1=u2sb[:, :, None, rq * 4 : (rq + 1) * 4].to_broadcast(
                        (P, IC, B, 4)
                    ),
                    op=mybir.AluOpType.add,
                )

            # --- rbbar[b, r] + bias as 4 M=1 rows on partition 0 (cols
            # b*32+r), emitted HERE so these tiny matmuls run in the PE
            # bubble between the V transposes and the hft-gated out phase
            # (their result is only consumed by the final ACT bias) ---
            ps_rb = ppool.tile([P, 512], fp32, tag="ps")
            for b in range(B):
                for jc in range(JC):
                    nc.tensor.matmul(
                        ps_rb[:1, b * 32 : b * 32 + RB],
                        hbbarT[:, jc * B + b : jc * B + b + 1],
                        u2sb[:, jc, RB : 2 * RB],
                        start=(jc == 0),
                        stop=False,
                    )
                nc.tensor.matmul(
                    ps_rb[:1, b * 32 : b * 32 + RB],
                    ones1[:1, :1],
                    bias_sb,
                    start=False,
                    stop=True,
                )
            rbbF = dpool.tile([1, B * 32], fp32, tag="rbbF", bufs=3)
            nc.vector.tensor_copy(out=rbbF, in_=ps_rb[:1, : B * 32])
            # transpose the row to a per-partition column [128, 1] (parked
            # in spare columns of the same PSUM bank) so the final
            # PSUM->SBUF copy can fold it in as an ACT bias
            nc.tensor.transpose(ps_rb[:P, 384:385], rbbF, identity_sq[:1, :1])
            rbbC = dpool.tile([P, 1], fp32, tag="rbbC", bufs=3)
            nc.vector.tensor_copy(out=rbbC, in_=ps_rb[:P, 384:385])

            # --- out[r, x]: contract i. The 4 b's live at PSUM base
            # partitions {0,32,64,96} (col-tiled, concurrent chains); a
            # trailing K=1 ones-matmul folds in rbbar+bias per group.
            ps_o = ppool.tile([P, 512], fp32, tag="ps")
            for ic in range(IC):
                for b in range(B):
                    nc.tensor.matmul(
                        ps_o[b * 32 : b * 32 + RB, :S],
                        vass[:, ic, b, :],
                        hft_tiles[b][:, ic, :],
                        start=(ic == 0),
                        stop=(ic == IC - 1),
                        tile_position=(0, b * 32),
                    )
            out_sb = dpool.tile([P, S], fp32, tag="out_sb", bufs=3)
            nc.scalar.activation(
                out=out_sb,
                in_=ps_o,
                func=mybir.ActivationFunctionType.Identity,
                bias=rbbC,
            )
            for b in range(B):
                nc.scalar.dma_start(
                    out=out_d.ap()[b], in_=out_sb[b * 32 : b * 32 + RB]
                )


def _get_nc(n_repeat=1):
    if n_repeat not in _NC_CACHE:
        _NC_CACHE[n_repeat] = _build_nc(n_repeat)
    return _NC_CACHE[n_repeat]


def _prep_inputs(h_forward, h_backward, U_1, U_2, bias):
    import ml_dtypes

    bf16 = ml_dtypes.bfloat16
    f8 = ml_dtypes.float8_e3m4
    hf = np.ascontiguousarray(np.asarray(h_forward, dtype=np.float32))
    hb = np.ascontiguousarray(np.asarray(h_backward, dtype=np.float32))
    u1 = np.asarray(U_1, dtype=np.float32)
    u2 = np.asarray(U_2, dtype=np.float32)
    bz = np.asarray(bias, dtype=np.float32)

    hft = np.ascontiguousarray(
        (hf.transpose(0, 2, 1) * np.float32(1.0 / VSCALE)).astype(bf16)
    )  # [B, i, x], scaled so out = hft' @ (VSCALE*(V+U2a)) is exact

    in_maps = []
    for c in range(NCORES):
        rs = slice(c * RB, (c + 1) * RB)
        u1t_c = np.ascontiguousarray(
            (u1[:, rs, :].transpose(2, 1, 0) * np.float32(U1_SCALE)).astype(f8)
        )  # [j, r, i], fp8 with power-of-2 gain to dodge denormals
        # pre-packed u2sb layout [d%P, dchunk, 2*RB]: cols 0:RB = VSCALE*U2a,
        # RB:2RB = U2b/(S*HB_SCALE) (so both match their scaled partners)
        u2t_c = np.ascontiguousarray(
            np.concatenate(
                [
                    u2[:D, rs].reshape(IC, P, RB).transpose(1, 0, 2)
                    * np.float32(VSCALE),
                    u2[D:, rs].reshape(IC, P, RB).transpose(1, 0, 2)
                    * np.float32(1.0 / (S * HB_SCALE)),
                ],
                axis=2,
            )
        )
        bias_c = np.ascontiguousarray(bz[rs].reshape(1, RB))
        hb_c = np.ascontiguousarray(
            (hb[:, c * SY : (c + 1) * SY, :].transpose(2, 0, 1)
             * np.float32(HB_SCALE)).astype(f8)
        )  # [D(j), B, SY], fp8 e3m4 at x2 (normal range)
        in_maps.append(
            {
                "hft": hft,
                "hb": hb_c,
                "u1t": u1t_c,
                "u2t": u2t_c,
                "biasr": bias_c,
            }
        )
    return in_maps


def _get_exec():
    """One jitted sharded executable, cached for the process lifetime.

    Repeated kernel() calls reuse it — re-jitting a second executable with
    collectives in the same process has been observed to wedge the NRT
    (NRT_EXEC_UNIT_UNRECOVERABLE), while re-executing one executable is solid.
    """
    if "exec" in _EXEC_CACHE:
        return _EXEC_CACHE["exec"]

    import jax
    from jax.sharding import Mesh, PartitionSpec

    import warnings

    with warnings.catch_warnings():
        warnings.simplefilter("ignore")
        from jax.experimental.shard_map import shard_map

    from concourse import mybir
    from concourse.bass2jax import (
        _bass_exec_p,
        install_neuronx_cc_hook,
        partition_id_tensor,
    )

    install_neuronx_cc_hook()
    nc = _get_nc()
    partition_name = nc.partition_id_tensor.name if nc.partition_id_tensor else None
    in_names, out_names, out_avals = [], [], []
    for alloc in nc.m.functions[0].allocations:
        if not isinstance(alloc, mybir.MemoryLocationSet):
            continue
        name = alloc.memorylocations[0].name
        if alloc.kind == "ExternalInput":
            if name != partition_name:
                in_names.append(name)
        elif alloc.kind == "ExternalOutput":
            out_names.append(name)
            out_avals.append(
                jax.core.ShapedArray(tuple(alloc.tensor_shape), mybir.dt.np(alloc.dtype))
            )
    all_names = in_names + out_names
    if partition_name is not None:
        all_names = all_names + [partition_name]

    def _body(*args):
        operands = list(args)
        if partition_name is not None:
            operands.append(partition_id_tensor())
        return tuple(
            _bass_exec_p.bind(
                *operands,
                out_avals=tuple(out_avals),
                in_names=tuple(all_names),
                out_names=tuple(out_names),
                lowering_input_output_aliases=(),
                sim_require_finite=True,
                sim_require_nnan=True,
                nc=nc,
            )
        )

    devices = jax.devices()[:NCORES]
    mesh = Mesh(np.asarray(devices), ("core",))
    n_args = len(in_names) + len(out_avals)
    fn = jax.jit(
        shard_map(
            _body,
            mesh=mesh,
            in_specs=(PartitionSpec("core"),) * n_args,
            out_specs=(PartitionSpec("core"),) * len(out_names),
            check_rep=False,
        ),
        keep_unused=True,
    )
    sh = jax.sharding.NamedSharding(mesh, PartitionSpec("core"))
    _EXEC_CACHE["exec"] = (fn, sh, in_names, out_names, out_avals)
    return _EXEC_CACHE["exec"]


_EXEC_CACHE = {}


def kernel(h_forward, h_backward, U_1, U_2, bias):
    import jax

    fn, sh, in_names, out_names, out_avals = _get_exec()
    in_maps = _prep_inputs(h_forward, h_backward, U_1, U_2, bias)
    args = [
        jax.device_put(
            np.concatenate([in_maps[c][name] for c in range(NCORES)], axis=0), sh
        )
        for name in in_names
    ]
    for av in out_avals:
        args.append(
            jax.device_put(
                np.zeros((NCORES * av.shape[0], *av.shape[1:]), av.dtype), sh
            )
        )
    out_arrs = fn(*args)
    oi = out_names.index("out")
    full = np.asarray(out_arrs[oi]).reshape(NCORES, B, RB, S)  # [core, B, RB, S]
    out = np.concatenate(list(full), axis=1)  # [B, R, S]
    return np.ascontiguousarray(out.transpose(0, 2, 1))  # [B, S, R]

